# revision 1
# baseline (speedup 1.0000x reference)
"""Trainium2 Bass kernel for GQA attention (nn_Attention_50053548868012).

Math (reference):
  q = einsum('bsm,mrkh->brksh', x, wq);  k = x@wk;  v = x@wv        (per kv head)
  RoPE on q, k (k also scaled by H^-0.5), causal-masked softmax(q k^T),
  y = a @ v, out = einsum('brksh,rkhm->bsm', y, wo)

Sharding: tensor-parallel over the KV-head axis — core c owns kv head c
(its 4 query heads, wk/wv column slices, and a 512-wide slice of wo's
output dim). yT is AllGathered (chunked along seq) so each core computes
a 512-column slice of the output projection with the full 4096-dim
contraction. Host concatenates the 8 output slices.

Performance notes (measured on these axon trn2 cores):
 - matmuls cost ~0.65-0.9us each nearly independent of dtype; mm COUNT is
   what matters -> Z rows are summed on DVE/GPSIMD instead of PE.
 - DMA is ~60-80GB/s with 2KB per-partition lines but ~300+GB/s with 8KB
   lines -> all big tensors are host-packed so every large DMA moves
   [128, >=2048 f32] slabs; phase 1 runs m-blocks of 8 with SBUF
   accumulation so x slabs can be full 8KB-line reads.
 - float32r matmuls (same speed as bf16 here) with the whole operand
   chain declared f32r (walrus requires producers to round to f32r).
"""

import numpy as np

import concourse.bass as bass
import concourse.tile as tile
from concourse import bacc, mybir
from concourse.bass_utils import run_bass_kernel_spmd
from concourse.masks import make_identity

NCORES = 8
S = 2048
MD = 4096
H = 128
R = 4
KV = 8
PT = 128          # partition tile
SC = 512          # free-dim chunk
RH = R * H        # 512
MB = 8            # m-tiles per phase-1 block
SCALE = float(H) ** -0.5
NEG = -30000.0

f32 = mybir.dt.float32
f32r = mybir.dt.float32r


def build_bass(s=S, collective=True, phases=3, reps=1):
    nc = _emit(s, collective, phases, reps)
    nc.compile()
    return nc


def _emit(s, collective, phases, reps=1):
    assert s % SC == 0
    n_sc = s // SC          # seq chunks
    n_mt = MD // PT         # model-dim tiles (32)
    n_tt = s // PT          # seq tiles of 128
    tpc = SC // PT          # 128-tiles per chunk (4)
    n_blk = n_mt // MB      # phase-1 m-blocks (4)
    hh = H // 2

    nc = bacc.Bacc("TRN2", target_bir_lowering=False, debug=False,
                   num_devices=NCORES)

    # host-packed layouts: partition-major so big DMAs have 8KB+ lines
    xT = nc.dram_tensor("xT", [PT, n_mt, s], f32r, kind="ExternalInput").ap()
    wq = nc.dram_tensor("wq", [PT, n_mt, RH], f32r, kind="ExternalInput").ap()
    wk = nc.dram_tensor("wk", [PT, n_mt, H], f32r, kind="ExternalInput").ap()
    wv = nc.dram_tensor("wv", [PT, n_mt, H], f32r, kind="ExternalInput").ap()
    wo = nc.dram_tensor("wo", [PT, R, MD], f32r, kind="ExternalInput").ap()
    cosT = nc.dram_tensor("cosT", [H, s], f32, kind="ExternalInput").ap()
    sinT = nc.dram_tensor("sinT", [H, s], f32, kind="ExternalInput").ap()
    mask4 = nc.dram_tensor("mask4", [PT, tpc * SC], f32,
                           kind="ExternalInput").ap()
    outp = nc.dram_tensor("outp", [PT, n_tt, MD], f32,
                          kind="ExternalOutput").ap()

    with tile.TileContext(nc) as tc:
      for _rep in range(reps):
        with tc.tile_pool(name="const", bufs=1) as const_pool, \
             tc.tile_pool(name="dram", bufs=1, space="DRAM") as dram_pool:
            ones_f = const_pool.tile([PT, PT], f32)
            nc.gpsimd.memset(ones_f[:], 1.0)
            ones_sb = const_pool.tile([PT, PT], f32r)
            nc.scalar.copy(ones_sb[:], ones_f[:])


            ypersist_pool = tc.alloc_tile_pool(name="ypersist", bufs=1)
            yT_sb = ypersist_pool.tile([H, R, s], f32r)
            with tc.tile_pool(name="qkv", bufs=1) as qkv_pool:
                qT_sb = qkv_pool.tile([H, R, s], f32r)
                kT_sb = qkv_pool.tile([H, s], f32r)
                v_sb = qkv_pool.tile([PT, n_tt, H], f32r)

                # ---------- Phase 1: projections (m-blocked) + RoPE ----------
                with tc.tile_pool(name="ph1", bufs=1) as ph1_pool, \
                     tc.tile_pool(name="p1ps", bufs=1, space="PSUM") as p1_psum, \
                     tc.tile_pool(name="tpps", bufs=2, space="PSUM") as tp_psum:
                    w_ctx = tc.tile_pool(name="w1", bufs=1)
                    w_pool = w_ctx.__enter__()
                    x_ctx = tc.tile_pool(name="xslab", bufs=1)
                    x_pool = x_ctx.__enter__()
                    vT_sb = ph1_pool.tile([H, s], f32)
                    ident = ph1_pool.tile([PT, PT], f32)
                    make_identity(nc, ident[:])
                    # consts over SWDGE to keep HWDGE free for weight/x slabs
                    cos_sb = ph1_pool.tile([H, s], f32)
                    nc.gpsimd.dma_start(cos_sb[:], cosT)
                    sin_sb = ph1_pool.tile([H, s], f32)
                    nc.gpsimd.dma_start(sin_sb[:], sinT)

                    accs = {}
                    for j in range(R):
                        accs[j] = lambda ssl, j=j: qT_sb[:, j, ssl]
                    accs[R] = lambda ssl: kT_sb[:, ssl]
                    accs[R + 1] = lambda ssl: vT_sb[:, ssl]

                    for blk in range(n_blk):
                        wq_b = w_pool.tile([PT, MB, RH], f32r, tag="wqb")
                        nc.sync.dma_start(
                            wq_b[:], wq[:, blk * MB:(blk + 1) * MB, :])
                        wk_b = w_pool.tile([PT, MB, H], f32r, tag="wkb")
                        nc.sync.dma_start(
                            wk_b[:], wk[:, blk * MB:(blk + 1) * MB, :])
                        wv_b = w_pool.tile([PT, MB, H], f32r, tag="wvb")
                        nc.sync.dma_start(
                            wv_b[:], wv[:, blk * MB:(blk + 1) * MB, :])
                        xs = []
                        for ml in range(MB):
                            xsl = x_pool.tile([PT, s], f32r, tag=f"x{ml}",
                                              name="xsl")
                            nc.sync.dma_start(
                                xsl[:], xT[:, blk * MB + ml, :])
                            xs.append(xsl)
                        for sc_i in range(n_sc):
                            ssl = slice(sc_i * SC, (sc_i + 1) * SC)
                            ps6 = [p1_psum.tile([PT, SC], f32, tag=f"pa{u}",
                                                name=f"ps6_{u}")
                                   for u in range(R + 2)]
                            for ml in range(MB):
                                rx = xs[ml][:, ssl]
                                st = ml == 0
                                sp = ml == MB - 1
                                for j in range(R):
                                    nc.tensor.matmul(
                                        ps6[j][:],
                                        wq_b[:, ml, j * H:(j + 1) * H],
                                        rx, start=st, stop=sp)
                                nc.tensor.matmul(
                                    ps6[R][:], wk_b[:, ml, :], rx,
                                    start=st, stop=sp)
                                nc.tensor.matmul(
                                    ps6[R + 1][:], wv_b[:, ml, :], rx,
                                    start=st, stop=sp)
                            # spill/accumulate into SBUF (frees banks fast)
                            for u in range(R + 2):
                                acc = accs[u](ssl)
                                if blk == 0:
                                    nc.scalar.copy(acc, ps6[u][:])
                                else:
                                    nc.vector.tensor_add(
                                        acc, ps6[u][:], acc)

                    # RoPE in place on qT/kT. The half-rotation is done with
                    # SBUF->SBUF DMAs (engines can't mix SB base partitions),
                    # then three whole-tensor base-aligned DVE ops.
                    x_ctx.__exit__(None, None, None)
                    w_ctx.__exit__(None, None, None)
                    rope_ctx = tc.tile_pool(name="rope", bufs=1)
                    rope_pool = rope_ctx.__enter__()
                    qsw = rope_pool.tile([H, R, s], f32r, tag="qsw", bufs=1)
                    ksw = rope_pool.tile([H, s], f32r, tag="ksw", bufs=1)
                    nc.sync.dma_start(qsw[0:hh, :, :], qT_sb[hh:H, :, :])
                    nc.sync.dma_start(qsw[hh:H, :, :], qT_sb[0:hh, :, :])
                    nc.sync.dma_start(ksw[0:hh, :], kT_sb[hh:H, :])
                    nc.sync.dma_start(ksw[hh:H, :], kT_sb[0:hh, :])
                    sin_q = sin_sb[:, None, :].broadcast_to([H, R, s])
                    cos_q = cos_sb[:, None, :].broadcast_to([H, R, s])
                    nc.vector.tensor_mul(qsw[:], qsw[:], sin_q)
                    nc.vector.tensor_mul(qT_sb[:], qT_sb[:], cos_q)
                    nc.vector.tensor_add(qT_sb[:], qT_sb[:], qsw[:])
                    nc.vector.tensor_mul(ksw[:], ksw[:], sin_sb[:])
                    nc.vector.tensor_mul(kT_sb[:], kT_sb[:], cos_sb[:])
                    nc.vector.tensor_add(kT_sb[:], kT_sb[:], ksw[:])
                    for tt in range(n_tt):
                        ps_t = tp_psum.tile([PT, PT], f32, tag="tp",
                                            name="ps_t")
                        nc.tensor.transpose(
                            ps_t[:], vT_sb[:, tt * PT:(tt + 1) * PT],
                            ident[:])
                        nc.scalar.copy(v_sb[:, tt, :], ps_t[:])
                    rope_ctx.__exit__(None, None, None)

                if phases >= 2:
                    # ---------------- Phase 3 weights prefetch -------------------
                    with tc.tile_pool(name="w3", bufs=1) as w3_pool:
                        mask_sb = w3_pool.tile([PT, tpc, SC], f32)
                        nc.gpsimd.dma_start(
                            mask_sb[:], mask4.rearrange("p (j c) -> p j c", j=tpc))

                        # ---------------- Phase 2: attention ---------------------
                        with tc.tile_pool(name="epool", bufs=3) as e_pool, \
                             tc.tile_pool(name="zpool", bufs=2) as z_pool, \
                             tc.tile_pool(name="p2ps", bufs=2, space="PSUM") as p2_psum:
                            pending_fin = []
                            for c in range(n_sc):
                                T = (c + 1) * tpc
                                csl = slice(c * SC, (c + 1) * SC)
                                for j in range(R):
                                    ps_y = p2_psum.tile([H, SC], f32, tag="py")
                                    ps_z = p2_psum.tile([1, SC], f32, tag="pz",
                                                        bufs=1, name="ps_z")
                                    rq = qT_sb[:, j, csl]
                                    es = {}

                                    def qk_exp(p, rq=rq, T=T, es=es):
                                        # two score tiles in one 2-bank PSUM
                                        # tile; one mask add + one exp per pair
                                        t0 = 2 * p
                                        ps_s = p2_psum.tile([PT, 2 * SC], f32,
                                                            tag="ps", bufs=2,
                                                            name="ps_s")
                                        nc.tensor.matmul(
                                            ps_s[:, 0:SC],
                                            kT_sb[:, t0 * PT:(t0 + 1) * PT],
                                            rq, start=True, stop=True)
                                        nc.tensor.matmul(
                                            ps_s[:, SC:2 * SC],
                                            kT_sb[:, (t0 + 1) * PT:
                                                  (t0 + 2) * PT],
                                            rq, start=True, stop=True)
                                        jj = t0 - (T - tpc)
                                        if jj >= 0:
                                            nc.vector.tensor_add(
                                                ps_s[:].rearrange(
                                                    "q (a b) -> q a b", a=2),
                                                ps_s[:].rearrange(
                                                    "q (a b) -> q a b", a=2),
                                                mask_sb[:, jj:jj + 2, :])
                                        e_t = e_pool.tile([PT, 2 * SC], f32r,
                                                          tag="e", name="e_t")
                                        nc.scalar.activation(
                                            e_t[:], ps_s[:],
                                            mybir.ActivationFunctionType.Exp,
                                            scale=SCALE)
                                        es[p] = e_t

                                    P2 = T // 2
                                    qk_exp(0)
                                    # previous (c,j) normalization runs while
                                    # our QK prologue keeps the PE busy
                                    while pending_fin:
                                        pending_fin.pop(0)()
                                    for p in range(P2):
                                        if p + 1 < P2:
                                            qk_exp(p + 1)
                                        e_t = es.pop(p)
                                        for half in range(2):
                                            t = 2 * p + half
                                            esl = slice(half * SC,
                                                        (half + 1) * SC)
                                            nc.tensor.matmul(
                                                ps_y[:], v_sb[:, t, :],
                                                e_t[:, esl],
                                                start=(t == 0),
                                                stop=(t == T - 1))
                                            nc.tensor.matmul(
                                                ps_z[:], ones_sb[:, 0:1],
                                                e_t[:, esl],
                                                start=(t == 0),
                                                stop=(t == T - 1))

                                    def finalize(c=c, j=j, ps_y=ps_y, ps_z=ps_z,
                                                 csl=csl):
                                        rz = z_pool.tile([1, SC], f32r, tag="rz",
                                                         name="rz")
                                        with nc.allow_low_precision(
                                                reason="f32r is full-width"):
                                            nc.vector.reciprocal(rz[:], ps_z[:])
                                        ps_b = p2_psum.tile([PT, SC], f32,
                                                            tag="pb", bufs=1,
                                                            name="ps_b")
                                        nc.tensor.matmul(
                                            ps_b[:], ones_sb[0:1, :], rz[:],
                                            start=True, stop=True)
                                        b_sb = z_pool.tile([PT, SC], f32,
                                                           tag="bsb", name="b_sb")
                                        nc.scalar.copy(b_sb[:], ps_b[:])
                                        nc.vector.tensor_mul(
                                            yT_sb[:, j, csl], ps_y[:], b_sb[:])

                                    pending_fin.append(finalize)
                            while pending_fin:
                                pending_fin.pop(0)()

            # -------- Phase 3: local-rh output projection (host sums) -------
            if phases >= 3:
              with tc.tile_pool(name="w3b", bufs=1) as w3b_pool, \
                 tc.tile_pool(name="osb", bufs=2) as o_pool, \
                 tc.tile_pool(name="p3ps", bufs=1, space="PSUM") as p3_psum:
                wo_sb = w3b_pool.tile([PT, R, MD], f32r)
                for rl in range(R):
                    nc.sync.dma_start(wo_sb[:, rl, :], wo[:, rl, :])
                n_mc = MD // RH
                for st in range(n_tt):
                    o_acc = o_pool.tile([PT, MD], f32, tag="oacc",
                                        name="o_acc")
                    for mc in range(n_mc):
                        ps_o = p3_psum.tile([PT, RH], f32, tag=f"o{mc % 4}",
                                            bufs=2, name="ps_o")
                        for rl in range(R):
                            nc.tensor.matmul(
                                ps_o[:],
                                yT_sb[:, rl, st * PT:(st + 1) * PT],
                                wo_sb[:, rl, mc * RH:(mc + 1) * RH],
                                start=(rl == 0), stop=(rl == R - 1))
                        nc.scalar.copy(
                            o_acc[:, mc * RH:(mc + 1) * RH], ps_o[:])
                    nc.sync.dma_start(outp[:, st, :], o_acc[:])
            ypersist_pool.release()
    return nc


def make_mask4():
    """mask4[:, 512j:512(j+1)][ti, sj] = 0 if 128j+ti <= sj else NEG."""
    tpc = SC // PT
    m = np.full((PT, tpc * SC), NEG, dtype=np.float32)
    for j in range(tpc):
        ti = np.arange(PT)[:, None]
        sj = np.arange(SC)[None, :]
        m[:, j * SC:(j + 1) * SC] = np.where(128 * j + ti <= sj, 0.0, NEG)
    return m


def _pack_pm(a):
    """[n_mt*128, C] -> [128, n_mt, C] (partition-major for 8KB DMA lines)."""
    n_mt = a.shape[0] // PT
    return np.ascontiguousarray(
        a.reshape(n_mt, PT, a.shape[1]).transpose(1, 0, 2))


def shard_inputs(x, wq, wk, wv, wo, mask, sin, cos, s=S):
    """Build the 8 per-core input maps from the full problem inputs."""
    del mask  # causality is hardcoded (mask4 tiles)
    xTp = _pack_pm(np.ascontiguousarray(
        np.asarray(x, dtype=np.float32).reshape(s, MD).T))
    cosT = np.ascontiguousarray(np.asarray(cos, dtype=np.float32).T)
    sinT = np.ascontiguousarray(np.asarray(sin, dtype=np.float32).T)
    sign = np.concatenate(
        [-np.ones((H // 2, 1)), np.ones((H // 2, 1))]).astype(np.float32)
    sinTs = np.ascontiguousarray(sinT * sign)
    wo = np.asarray(wo, dtype=np.float32)
    mask4 = make_mask4()
    wq = np.asarray(wq, dtype=np.float32)
    wk = np.asarray(wk, dtype=np.float32)
    wv = np.asarray(wv, dtype=np.float32)
    in_maps = []
    for c in range(NCORES):
        in_maps.append({
            "xT": xTp,
            "wq": _pack_pm(np.ascontiguousarray(
                wq[:, :, c, :].reshape(MD, RH))),
            "wk": _pack_pm(np.ascontiguousarray(wk[:, c, :])),
            "wv": _pack_pm(np.ascontiguousarray(wv[:, c, :])),
            "wo": _pack_pm(np.ascontiguousarray(
                wo[:, c, :, :].reshape(RH, MD))),
            "cosT": cosT,
            "sinT": sinTs,
            "mask4": mask4,
        })
    return in_maps


def unpack_out(outp_arr, s=S):
    """[128, s/128, MD] -> [s, MD]."""
    return np.ascontiguousarray(
        np.asarray(outp_arr).reshape(PT, s // PT, MD).transpose(
            1, 0, 2).reshape(s, MD))


_NC_CACHE = {}


def kernel(x, wq, wk, wv, wo, mask, sin, cos):
    s = x.shape[1]
    if s not in _NC_CACHE:
        _NC_CACHE[s] = build_bass(s)
    nc = _NC_CACHE[s]
    in_maps = shard_inputs(x, wq, wk, wv, wo, mask, sin, cos, s=s)
    res = run_bass_kernel_spmd(nc, in_maps, list(range(NCORES)))
    out = unpack_out(res.results[0]["outp"], s)
    for c in range(1, NCORES):
        out = out + unpack_out(res.results[c]["outp"], s)
    return out.reshape(1, s, MD).astype(np.float32)



# revision 8
# speedup vs baseline: 85.2194x; 85.2194x over previous
"""Trainium2 Bass kernel for GQA attention (nn_Attention_50053548868012).

Math (reference):
  q = einsum('bsm,mrkh->brksh', x, wq);  k = x@wk;  v = x@wv   (per kv head)
  RoPE on q, k; causal-masked softmax(q k^T / sqrt(H)); y = a @ v;
  out = einsum('brksh,rkhm->bsm', y, wo)

Sharding: tensor-parallel over the KV-head axis - core c owns kv head c
(its 4 query heads, wk/wv column slices, and the 512-row slice of wo).
Each core computes its partial full-size output; the host sums the 8
partial outputs (the all-reduce).

v2 design (vs the phase-separated v1 baseline):
 - everything bf16 (rel err ~7e-3 vs the 2e-2 gate): halves DMA and SBUF
   so all weights stay resident and x is read exactly once.
 - chunk-major fusion: for each 512-seq chunk, projections accumulate
   32-deep in PSUM (no DVE spill-adds), RoPE runs on DVE under the next
   chunk's PE work, then attention + the output projection for the chunk
   keep the PE stream dense end-to-end.
 - softmax denominator: e-tiles are accumulated on DVE (bf16 2x) and
   reduced with ONE ones-matmul per (chunk, head) instead of a second
   full matmul pipe on the PE (-80k PE rows).
 - exp is the only table function on ACT; spills/copies shared between
   ACT engine queues so no engine's in-order queue blocks another
   segment's critical path.
"""

import numpy as np
import ml_dtypes

import concourse.bass as bass
import concourse.tile as tile
from concourse import bacc, mybir
from concourse.bass_utils import run_bass_kernel_spmd
from concourse.masks import make_identity

NCORES = 8
S = 2048
MD = 4096
H = 128
R = 4
KV = 8
PT = 128           # partition tile
SC = 512           # seq chunk = matmul free dim
NMT = MD // PT     # 32 model-dim tiles
MB = 8             # m-tiles per x/wq slab
NSL = NMT // MB    # 4 slabs
TPC = SC // PT     # 4 seq-tiles per chunk
HH = H // 2
RH = R * H         # 512
SCALE = float(H) ** -0.5
NEG = -30000.0

f32 = mybir.dt.float32
bf16 = mybir.dt.bfloat16
BF_NP = ml_dtypes.bfloat16

EXP = mybir.ActivationFunctionType.Exp


def build_bass(s=S, collective=True, phases=3, reps=1):
    nc = _emit(s, phases, reps)
    nc.compile()
    return nc


def _emit(s, phases, reps):
    n_sc = s // SC
    n_tt = s // PT
    nc = bacc.Bacc("TRN2", target_bir_lowering=False, debug=False,
                   num_devices=NCORES)

    xc = nc.dram_tensor("xc", [PT, n_sc, NMT, SC], bf16,
                        kind="ExternalInput").ap()
    wqd = nc.dram_tensor("wq", [PT, NSL, MB, RH], bf16,
                         kind="ExternalInput").ap()
    wkd = nc.dram_tensor("wk", [PT, NMT, H], bf16, kind="ExternalInput").ap()
    wvd = nc.dram_tensor("wv", [PT, NMT, H], bf16, kind="ExternalInput").ap()
    wod = nc.dram_tensor("wo", [PT, R, MD], bf16, kind="ExternalInput").ap()
    cosd = nc.dram_tensor("cosT", [H, s], bf16, kind="ExternalInput").ap()
    sind = nc.dram_tensor("sinT", [H, s], bf16, kind="ExternalInput").ap()
    maskd = nc.dram_tensor("mask4", [PT, TPC * SC], f32,
                           kind="ExternalInput").ap()
    outp = nc.dram_tensor("outp", [PT, n_tt, MD], bf16,
                          kind="ExternalOutput").ap()

    with tile.TileContext(nc) as tc:
      for _rep in range(reps):
        with tc.tile_pool(name="const", bufs=1) as cpool, \
             tc.tile_pool(name="wts", bufs=1) as wpool, \
             tc.tile_pool(name="seqst", bufs=1) as spool, \
             tc.tile_pool(name="xslab", bufs=5) as xpool, \
             tc.tile_pool(name="qy", bufs=2) as qypool, \
             tc.tile_pool(name="ep", bufs=4) as epool, \
             tc.tile_pool(name="small", bufs=2) as smpool, \
             tc.tile_pool(name="oacc", bufs=2) as opool:

            # ---------------- weights + consts ----------------
            wq_sb = wpool.tile([PT, NMT, RH], bf16)
            wk_sb = wpool.tile([PT, NMT, H], bf16)
            wv_sb = wpool.tile([PT, NMT, H], bf16)
            wo_sb = wpool.tile([PT, R, MD], bf16)
            nc.gpsimd.dma_start(wk_sb[:], wkd)
            nc.gpsimd.dma_start(wv_sb[:], wvd)
            cos_sb = cpool.tile([H, s], bf16)
            nc.gpsimd.dma_start(cos_sb[:], cosd)
            sin_sb = cpool.tile([H, s], bf16)
            nc.gpsimd.dma_start(sin_sb[:], sind)
            mask_sb = cpool.tile([PT, TPC, SC], f32)
            nc.gpsimd.dma_start(
                mask_sb[:], maskd.rearrange("p (j c) -> p j c", j=TPC))
            nc.gpsimd.dma_start(wo_sb[:], wod)
            ones_bf = cpool.tile([PT, PT], bf16)
            nc.gpsimd.memset(ones_bf[:], 1.0)
            ident = cpool.tile([PT, PT], bf16)
            make_identity(nc, ident[:])

            kT_sb = spool.tile([H, s], bf16)
            v_sb = spool.tile([PT, n_tt, H], bf16)

            pend = []  # deferred (z / finalize) emitters

            def drain_one():
                if pend:
                    pend.pop(0)()

            def drain_all():
                while pend:
                    pend.pop(0)()

            # ---------------- phase 1: projections + RoPE ----------------
            def ph1(c, xs=None):
                csl = slice(c * SC, (c + 1) * SC)
                with tc.tile_pool(name=f"p1ps{c}", bufs=1,
                                  space="PSUM") as pp, \
                     tc.tile_pool(name=f"tpps{c}", bufs=2,
                                  space="PSUM") as tpp:
                    ps_u = [pp.tile([PT, SC], f32, tag=f"u{u}",
                                    name=f"ps_u{u}") for u in range(R + 2)]
                    if xs is None:  # chunk 0: DMA x slabs + wq slabs now
                        xs = []
                        for sl in range(NSL):
                            xsl = xpool.tile([PT, MB, SC], bf16, tag="x",
                                             name=f"x{c}_{sl}")
                            nc.sync.dma_start(
                                xsl[:], xc[:, c, sl * MB:(sl + 1) * MB, :])
                            nc.sync.dma_start(
                                wq_sb[:, sl * MB:(sl + 1) * MB, :],
                                wqd[:, sl, :, :])
                            xs.append(xsl)
                    for sl in range(NSL):
                        for ml in range(MB):
                            m = sl * MB + ml
                            rx = xs[sl][:, ml, :]
                            st = m == 0
                            sp = m == NMT - 1
                            nc.tensor.matmul(ps_u[R][:], wk_sb[:, m, :], rx,
                                             start=st, stop=sp)
                            nc.tensor.matmul(ps_u[R + 1][:], wv_sb[:, m, :],
                                             rx, start=st, stop=sp)
                            for j in range(R):
                                nc.tensor.matmul(
                                    ps_u[j][:],
                                    wq_sb[:, m, j * H:(j + 1) * H], rx,
                                    start=st, stop=sp)
                        drain_one()
                    qT_c = qypool.tile([H, R, SC], bf16, tag="qt",
                                       name=f"qT{c}")
                    vT_c = smpool.tile([H, SC], bf16, tag="vt",
                                       name=f"vT{c}")
                    for j in range(R):
                        nc.scalar.copy(qT_c[:, j, :], ps_u[j][:])
                    nc.scalar.copy(kT_sb[:, csl], ps_u[R][:])
                    nc.scalar.copy(vT_c[:], ps_u[R + 1][:])
                    for tt in range(TPC):
                        ps_t = tpp.tile([PT, PT], bf16, tag="tp",
                                        name="ps_t")
                        nc.tensor.transpose(
                            ps_t[:], vT_c[:, tt * PT:(tt + 1) * PT], ident[:])
                        nc.scalar.copy(v_sb[:, c * TPC + tt, :], ps_t[:])
                # RoPE chunk c (DVE work overlaps later PE segments)
                qsw = smpool.tile([H, R, SC], bf16, tag="qsw", bufs=1,
                                  name=f"qsw{c}")
                ksw = smpool.tile([H, SC], bf16, tag="ksw", bufs=1,
                                  name=f"ksw{c}")
                nc.gpsimd.dma_start(qsw[0:HH, :, :], qT_c[HH:H, :, :])
                nc.gpsimd.dma_start(qsw[HH:H, :, :], qT_c[0:HH, :, :])
                nc.gpsimd.dma_start(ksw[0:HH, :], kT_sb[HH:H, csl])
                nc.gpsimd.dma_start(ksw[HH:H, :], kT_sb[0:HH, csl])
                sin_c = sin_sb[:, csl][:, None, :].broadcast_to([H, R, SC])
                cos_c = cos_sb[:, csl][:, None, :].broadcast_to([H, R, SC])
                nc.vector.tensor_mul(qsw[:], qsw[:], sin_c)
                nc.vector.tensor_mul(qT_c[:], qT_c[:], cos_c)
                nc.vector.tensor_add(qT_c[:], qT_c[:], qsw[:])
                nc.vector.tensor_mul(ksw[:], ksw[:], sin_sb[:, csl])
                nc.vector.tensor_mul(kT_sb[:, csl], kT_sb[:, csl],
                                     cos_sb[:, csl])
                nc.vector.tensor_add(kT_sb[:, csl], kT_sb[:, csl], ksw[:])
                return qT_c

            # ---- phase 3 as interleavable work items (PE filler) ----
            def ph3_items(cprev, yT_prev, aps):
                items = []
                oaccs = {}
                for ti in range(TPC):
                    st = cprev * TPC + ti
                    for mc in range(MD // SC):
                        def item(ti=ti, mc=mc, st=st, yT_prev=yT_prev,
                                 aps=aps):
                            if mc == 0:
                                oaccs[ti] = opool.tile(
                                    [PT, MD], bf16, tag="oa",
                                    name=f"oacc{st}")
                            o_acc = oaccs[ti]
                            ps_o = aps.tile([PT, SC], f32, tag="po",
                                            bufs=2, name="ps_o")
                            for rl in range(R):
                                nc.tensor.matmul(
                                    ps_o[:],
                                    yT_prev[:, rl, ti * PT:(ti + 1) * PT],
                                    wo_sb[:, rl, mc * SC:(mc + 1) * SC],
                                    start=(rl == 0), stop=(rl == R - 1))
                            osl = o_acc[:, mc * SC:(mc + 1) * SC]
                            if mc % 2 == 0:
                                nc.scalar.copy(osl, ps_o[:])
                            else:
                                nc.vector.tensor_copy(osl, ps_o[:])
                            if mc == MD // SC - 1:
                                nc.sync.dma_start(outp[:, st, :], o_acc[:])
                        items.append(item)
                return items

            # -------- A-segment: attention(c) x output-proj(c-1) --------
            def aseg(c, qT_c, yT_prev, xpf=None):
                T = (c + 1) * TPC
                P = T // 2
                yT_c = qypool.tile([H, R, SC], bf16, tag="yt",
                                   name=f"yT{c}")
                with tc.tile_pool(name=f"aps{c}", bufs=1,
                                  space="PSUM") as aps:
                    if xpf is not None:
                        prefetch(xpf)
                    items = (ph3_items(c - 1, yT_prev, aps)
                             if yT_prev is not None else [])
                    n_items = len(items)
                    total_pairs = R * P
                    pairs_done = 0
                    items_done = 0
                    for j in range(R):
                        ps_y = aps.tile([H, SC], f32, tag="y", bufs=1,
                                        name=f"psy{c}_{j}")
                        esum = smpool.tile([PT, SC], bf16, tag="es",
                                           name=f"es{c}_{j}")
                        es = {}

                        def qk_exp(p, j=j, qT_c=qT_c, T=T, es=es):
                            t0 = 2 * p
                            ps_s = aps.tile([PT, 2 * SC], f32, tag="s",
                                            bufs=2, name="ps_s")
                            nc.tensor.matmul(
                                ps_s[:, 0:SC],
                                kT_sb[:, t0 * PT:(t0 + 1) * PT],
                                qT_c[:, j, :], start=True, stop=True)
                            nc.tensor.matmul(
                                ps_s[:, SC:2 * SC],
                                kT_sb[:, (t0 + 1) * PT:(t0 + 2) * PT],
                                qT_c[:, j, :], start=True, stop=True)
                            drain_one()
                            jj = t0 - (T - TPC)
                            if jj >= 0:
                                nc.vector.tensor_add(
                                    ps_s[:].rearrange(
                                        "q (a b) -> q a b", a=2),
                                    ps_s[:].rearrange(
                                        "q (a b) -> q a b", a=2),
                                    mask_sb[:, jj:jj + 2, :])
                            e_t = epool.tile([PT, 2 * SC], bf16, tag="e",
                                             name="e_t")
                            nc.scalar.activation(e_t[:], ps_s[:], EXP,
                                                 scale=SCALE)
                            es[p] = e_t

                        qk_exp(0)
                        if P > 1:
                            qk_exp(1)
                        for p in range(P):
                            if p + 2 < P:
                                qk_exp(p + 2)
                            e_t = es.pop(p)
                            for half in range(2):
                                t = 2 * p + half
                                esl = slice(half * SC, (half + 1) * SC)
                                nc.tensor.matmul(
                                    ps_y[:], v_sb[:, t, :], e_t[:, esl],
                                    start=(t == 0), stop=(t == T - 1))
                                if t == 0:
                                    nc.vector.tensor_copy(esum[:],
                                                          e_t[:, esl])
                                else:
                                    nc.vector.tensor_add(esum[:], esum[:],
                                                         e_t[:, esl])
                            pairs_done += 1
                            target = (n_items * pairs_done) // total_pairs
                            while items_done < target:
                                items.pop(0)()
                                items_done += 1

                        def stage1(esum=esum, aps=aps):
                            ps_z = aps.tile([PT, SC], f32, tag="zb",
                                            bufs=1, name="ps_zb")
                            nc.tensor.matmul(ps_z[0:1, :], ones_bf[:, 0:1],
                                             esum[:], start=True, stop=True)
                            rz = smpool.tile([1, SC], bf16, tag="rz",
                                             name="rz")
                            with nc.allow_low_precision(
                                    reason="bf16 softmax denom, 2e-2 tol"):
                                nc.vector.reciprocal(rz[:], ps_z[0:1, :])
                            return rz

                        rz_box = []

                        def stage2(rz_box=rz_box, ps_y=ps_y, yT_c=yT_c,
                                   j=j, aps=aps):
                            ps_b = aps.tile([PT, SC], f32, tag="zb",
                                            bufs=1, name="ps_zb")
                            nc.tensor.matmul(ps_b[:], ones_bf[0:1, :],
                                             rz_box[0][:], start=True,
                                             stop=True)
                            b_sb = smpool.tile([PT, SC], f32, tag="bb",
                                               name="b_sb")
                            nc.scalar.copy(b_sb[:], ps_b[:])
                            nc.vector.tensor_mul(yT_c[:, j, :], ps_y[:],
                                                 b_sb[:])

                        pend.append(lambda s1=stage1, rb=rz_box: rb.append(
                            s1()))
                        pend.append(stage2)
                    drain_all()
                    while items:
                        items.pop(0)()
                return yT_c

            # -------- tail output projection (no attention partner) --------
            def ph3_tail(cprev, yT_prev):
                with tc.tile_pool(name="p3tail", bufs=1,
                                  space="PSUM") as aps:
                    items = ph3_items(cprev, yT_prev, aps)
                    for it in items:
                        it()

            _prefetched = {c: [] for c in range(n_sc)}

            def prefetch(c):
                for sl in range(NSL):
                    xsl = xpool.tile([PT, MB, SC], bf16, tag="x",
                                     name=f"x{c}_{sl}")
                    nc.sync.dma_start(
                        xsl[:], xc[:, c, sl * MB:(sl + 1) * MB, :])
                    _prefetched[c].append(xsl)

            qts = {}
            yts = {}
            qts[0] = ph1(0)
            prefetch(1)
            qts[1] = ph1(1, _prefetched[1])
            yts[0] = aseg(0, qts[0], None, xpf=2)
            qts[2] = ph1(2, _prefetched[2])
            yts[1] = aseg(1, qts[1], yts[0], xpf=3)
            qts[3] = ph1(3, _prefetched[3])
            yts[2] = aseg(2, qts[2], yts[1])
            yts[3] = aseg(3, qts[3], yts[2])
            ph3_tail(3, yts[3])
            drain_all()
    return nc


# ---------------------------------------------------------------------------
# host-side packing
# ---------------------------------------------------------------------------

def make_mask4():
    """mask4[:, 512j:512(j+1)][ti, sj] = 0 if 128j+ti <= sj else NEG."""
    m = np.full((PT, TPC * SC), NEG, dtype=np.float32)
    for j in range(TPC):
        ti = np.arange(PT)[:, None]
        sj = np.arange(SC)[None, :]
        m[:, j * SC:(j + 1) * SC] = np.where(128 * j + ti <= sj, 0.0, NEG)
    return m


def shard_inputs(x, wq, wk, wv, wo, mask, sin, cos, s=S):
    del mask  # causality hardcoded via mask4
    n_sc = s // SC
    xT = np.asarray(x, np.float32).reshape(s, MD).T  # [MD, s]
    xc = np.ascontiguousarray(
        xT.reshape(NMT, PT, n_sc, SC).transpose(1, 2, 0, 3)).astype(BF_NP)
    cosT = np.asarray(cos, np.float32).T.astype(BF_NP)
    sign = np.concatenate(
        [-np.ones((HH, 1)), np.ones((HH, 1))]).astype(np.float32)
    sinT = (np.asarray(sin, np.float32).T * sign).astype(BF_NP)
    cosT = np.ascontiguousarray(cosT)
    sinT = np.ascontiguousarray(sinT)
    mask4 = make_mask4()
    wq = np.asarray(wq, np.float32)
    wk = np.asarray(wk, np.float32)
    wv = np.asarray(wv, np.float32)
    wo = np.asarray(wo, np.float32)
    in_maps = []
    for c in range(NCORES):
        wqc = wq[:, :, c, :].reshape(MD, RH)             # [M, R*H]
        wqp = np.ascontiguousarray(
            wqc.reshape(NSL, MB, PT, RH).transpose(2, 0, 1, 3)).astype(BF_NP)
        wkp = np.ascontiguousarray(
            wk[:, c, :].reshape(NMT, PT, H).transpose(1, 0, 2)).astype(BF_NP)
        wvp = np.ascontiguousarray(
            wv[:, c, :].reshape(NMT, PT, H).transpose(1, 0, 2)).astype(BF_NP)
        wop = np.ascontiguousarray(
            wo[:, c, :, :].transpose(1, 0, 2)).astype(BF_NP)  # [H, R, MD]
        in_maps.append({
            "xc": xc, "wq": wqp, "wk": wkp, "wv": wvp, "wo": wop,
            "cosT": cosT, "sinT": sinT, "mask4": mask4,
        })
    return in_maps


def unpack_out(outp_arr, s=S):
    a = np.asarray(outp_arr).astype(np.float32)
    return a.reshape(PT, s // PT, MD).transpose(1, 0, 2).reshape(s, MD)


_NC_CACHE = {}


def kernel(x, wq, wk, wv, wo, mask, sin, cos):
    s = x.shape[1]
    if s not in _NC_CACHE:
        _NC_CACHE[s] = build_bass(s)
    nc = _NC_CACHE[s]
    in_maps = shard_inputs(x, wq, wk, wv, wo, mask, sin, cos, s=s)
    res = run_bass_kernel_spmd(nc, in_maps, list(range(NCORES)))
    out = unpack_out(res.results[0]["outp"], s)
    for c in range(1, NCORES):
        out = out + unpack_out(res.results[c]["outp"], s)
    return out.reshape(1, s, MD).astype(np.float32)
